# revision 22
# baseline (speedup 1.0000x reference)
"""Trainium2 Bass kernel for the PRADA GCN encoder (3x GCNConv message passing).

Math (matching the jax reference):
    src/dst = edges + self loops;  deg = indegree(dst);  dinv = rsqrt(deg)
    conv(x, W, b) = dinv_d * ((sum_{e: dst=d} dinv_src * x_src) @ W) + b
    h      = tanh(conv(x, W1, b1))
    mean   = conv(h, Wm, bm);  logvar = conv(h, Wv, bv)
    z      = noise * exp(0.5*logvar) + mean

Strategy (8 NeuronCores, single SPMD NEFF), v2 "project-first":
  - Both convs are computed as  aggregate(project(x)):  the projected node
    features (xw = (dinv*x)@W1 [64],  hw2 = hs@[Wm|Wv] [64]) are what gets
    gathered per edge, as bf16 rows padded to 256B (the 128B pad is never
    read by the aggregation matmuls).
  - Destination nodes sharded contiguously across cores (12544 rows/core).
    Edges grouped host-side into K=128-slot groups; each group targets one
    32-destination PSUM window of one 128-dst tile via a one-hot bf16
    S [128, 32] stationary matmul accumulating in PSUM.
  - Gathers use gpsimd.dma_gather (int16 indices) over the 4 quarters of a
    permuted global row space; each quarter is produced by its own chunked
    AllGather (Shared output) so collectives overlap the producing pass.
  - Uniform group skeleton across cores (max over the 8 cores) so one NEFF
    serves all cores; per-core variation lives in idx / S input data.
  - Epilogues keep everything SBUF-resident: noise preloaded once, outputs
    accumulated in SBUF and stored once, per-supertile DRAM stores for the
    collective inputs.
"""

import numpy as np

# ----------------------------------------------------------------- constants
TILE_D = 128     # destinations per tile (PSUM partition dim)
WIN = 32         # destinations per PSUM window (col-group granularity)
NWIN = TILE_D // WIN
K = 128          # slots per group (matmul contraction dim)

# problem config (graded problem; kernel.py must be self-contained)
N_NODES = 100000
N_EDGES = 1200000
IN_DIM, HID_DIM, LAT_DIM = 128, 64, 32
N_CORES = 8
GMAX = 8         # max groups (1024 idxs) per dma_gather call
DMA_SCRATCH = 2**14  # SWDGE ring: 1024 descs (16B each, per partition)
BANK_F32 = 512   # fp32 elements per 2KB PSUM bank; matmul start=True zeroes
                 # the whole bank (ZERO_REGION) => one agg tile per bank
ST_TILES = 6     # tiles per supertile (each agg tile owns a PSUM bank)
N_CHUNKS = 4     # src-space quarters (int16 gather index limit)
ROW = 128        # padded feature row: 128 bf16 = 256B (dma_gather minimum)


def _ceil_div(a, b):
    return -(-a // b)


# ============================================================ preprocessing
class Prep:
    pass


def preprocess(edge_index, n_nodes, cores, st_tiles=ST_TILES):
    """Build the uniform group skeleton + per-core idx/S tensors."""
    p = Prep()
    src = edge_index[0].astype(np.int64)
    dst = edge_index[1].astype(np.int64)

    npad = _ceil_div(n_nodes, cores * TILE_D) * cores * TILE_D
    dpc = npad // cores
    tiles_pc = dpc // TILE_D
    n_chunks = N_CHUNKS
    qsize = dpc // n_chunks          # rows per (core, quarter)
    assert dpc % n_chunks == 0
    chunk = qsize * cores            # rows per permuted global quarter
    assert chunk <= 32767
    n_st = _ceil_div(tiles_pc, st_tiles)

    # degree includes the self loop (+1); pad nodes get deg 1
    deg = np.bincount(dst, minlength=npad).astype(np.float32)
    deg[:n_nodes] += 1.0
    deg[n_nodes:] = 1.0
    dinv = (1.0 / np.sqrt(deg)).astype(np.float32)

    core_id = dst // dpc
    dloc = dst % dpc
    t = dloc // TILE_D
    w = (dloc % TILE_D) // WIN
    wcol = dloc % WIN
    # permuted src space: row r = dpc*k + m, m = 3136*q + j
    #   -> quarter q, local idx 3136*k + j  (quarter q = AllGather q's output)
    sk = src // dpc
    sm = src % dpc
    c = sm // qsize
    srcl = (qsize * sk + (sm % qsize)).astype(np.int16)

    # counts per (core, t, w, c) -> uniform skeleton G = ceil(max_cores/K)
    key = ((core_id * tiles_pc + t) * NWIN + w) * n_chunks + c
    counts = np.bincount(key, minlength=cores * tiles_pc * NWIN * n_chunks)
    counts = counts.reshape(cores, tiles_pc, NWIN, n_chunks)
    G = _ceil_div(counts.max(axis=0), K).astype(np.int64)  # [tiles_pc, NWIN, n_chunks]
    # every (t, w) needs >=1 group so its PSUM window gets written (start=True)
    empty_tw = G.sum(axis=2) == 0
    G[:, :, 0][empty_tw] = 1

    # enumerate groups in emission order: st -> chunk -> tile -> window -> g
    slot_off = np.zeros((tiles_pc, NWIN, n_chunks), np.int64)
    group_tile = []
    group_win = []
    segments = []  # per (st, c): dict
    gidx = 0
    soff = 0
    for s in range(n_st):
        ts = range(s * st_tiles, min((s + 1) * st_tiles, tiles_pc))
        for cc in range(n_chunks):
            g_lo, s_lo = gidx, soff
            for tt in ts:
                for ww in range(NWIN):
                    gg = int(G[tt, ww, cc])
                    slot_off[tt, ww, cc] = soff
                    group_tile.extend([tt] * gg)
                    group_win.extend([ww] * gg)
                    gidx += gg
                    soff += gg * K
            segments.append(dict(st=s, chunk=cc, g_lo=g_lo, g_hi=gidx,
                                 s_lo=s_lo, s_hi=soff))
    n_groups, n_slots = gidx, soff
    group_tile = np.asarray(group_tile, np.int64)
    group_win = np.asarray(group_win, np.int64)

    # first/last group flags per (t, w) in emission order
    first = np.zeros(n_groups, bool)
    last = np.zeros(n_groups, bool)
    seen = {}
    for g in range(n_groups):
        kk = (int(group_tile[g]), int(group_win[g]))
        if kk not in seen:
            first[g] = True
        seen[kk] = g
    for kk, g in seen.items():
        last[g] = True

    # ------- per-core slot assignment (vectorized rank within (core,t,w,c))
    order = np.lexsort((c, w, t, core_id))
    key_sorted = key[order]
    starts = np.r_[0, np.flatnonzero(np.diff(key_sorted)) + 1]
    group_start_of = np.zeros(len(key_sorted), np.int64)
    group_start_of[starts] = starts
    np.maximum.accumulate(group_start_of, out=group_start_of)
    rank = np.arange(len(key_sorted)) - group_start_of

    slot = slot_off[t[order], w[order], c[order]] + rank
    corev = core_id[order]

    idx16 = np.zeros((cores, n_slots), np.int16)
    idx16[corev, slot] = srcl[order]
    s8 = np.zeros((cores, 128, n_groups * 32), np.int8)
    s8[corev, slot % K, (slot // K) * 32 + wcol[order]] = 1

    # wrapped gather-index layout: idx i -> [i%16, i//16], replicated x8 rows
    wrapped = idx16.reshape(cores, -1, 16).transpose(0, 2, 1)  # [cores,16,S/16]
    wrapped = np.tile(wrapped, (1, 8, 1)).copy()               # [cores,128,S/16]

    p.npad, p.dpc, p.tiles_pc, p.n_st = npad, dpc, tiles_pc, n_st
    p.n_chunks, p.chunk, p.qsize, p.st_tiles = n_chunks, chunk, qsize, st_tiles
    p.n_groups, p.n_slots = n_groups, n_slots
    p.group_tile, p.group_win = group_tile, group_win
    p.group_first, p.group_last = first, last
    p.segments = segments
    p.dinv = dinv
    p.idx_wrapped = wrapped
    p.s8 = s8
    p.cores = cores
    return p


# ============================================================ bass program
def build_program(p, in_dim, hid_dim, lat_dim, has_b1, has_bmv,
                  no_collective=False, collective_mode="chunked_shared"):
    import concourse.bacc as bacc
    import concourse.mybir as mybir
    import concourse.tile as tile
    from concourse.library_config import mlp
    from concourse.masks import make_identity

    f32 = mybir.dt.float32
    bf16 = mybir.dt.bfloat16
    nc = bacc.Bacc("TRN2", target_bir_lowering=False, debug=False,
                   num_devices=p.cores, dynamic_dma_scratch_size=DMA_SCRATCH)

    lat2 = 2 * lat_dim
    n_st, st_tiles, tiles_pc = p.n_st, p.st_tiles, p.tiles_pc
    dpc, qsize = p.dpc, p.qsize

    # ---- inputs
    xT_t = nc.dram_tensor("xT", [in_dim, dpc], bf16, kind="ExternalInput")
    w1_t = nc.dram_tensor("w1", [in_dim, hid_dim], bf16, kind="ExternalInput")
    wmv_t = nc.dram_tensor("wmv", [hid_dim, lat2], bf16, kind="ExternalInput")
    idx_t = nc.dram_tensor("idx", [128, p.n_slots // 16], mybir.dt.int16,
                           kind="ExternalInput")
    sbf_t = nc.dram_tensor("sbf", [128, p.n_groups * 32], bf16,
                           kind="ExternalInput")
    dinv_t = nc.dram_tensor("dinv", [128, tiles_pc], f32, kind="ExternalInput")
    noise_t = nc.dram_tensor("noise", [128, tiles_pc * lat_dim], f32,
                             kind="ExternalInput")
    if has_b1:
        b1b_t = nc.dram_tensor("b1b", [128, hid_dim], f32, kind="ExternalInput")
    if has_bmv:
        bmvb_t = nc.dram_tensor("bmvb", [128, lat2], f32, kind="ExternalInput")

    # ---- output: per tile block [mean(32) | logvar(32) | z(32)]
    out_t = nc.dram_tensor("out", [128, tiles_pc * 3 * lat_dim], f32,
                           kind="ExternalOutput")

    segs_by_st = {}
    for seg in p.segments:
        segs_by_st.setdefault(seg["st"], []).append(seg)

    # quarter q's shard rows are complete after supertile fin_st[q]
    fin_st = [min(_ceil_div((q + 1) * qsize, st_tiles * TILE_D), n_st) - 1
              for q in range(p.n_chunks)]

    with tile.TileContext(nc) as tc:
        with (
            tc.tile_pool(name="const", bufs=1) as cpool,
            tc.tile_pool(name="sb", bufs=2) as sb,
            tc.tile_pool(name="ep", bufs=3) as ep,
            tc.tile_pool(name="ps", bufs=1, space="PSUM") as ps,
            tc.tile_pool(name="pse", bufs=1, space="PSUM") as pse,
            tc.tile_pool(name="dram", bufs=1, space="DRAM") as dram,
        ):
            nc.gpsimd.load_library(mlp)
            ident = cpool.tile([128, 128], f32)
            make_identity(nc, ident[:])
            w1_sb = cpool.tile([in_dim, hid_dim], bf16)
            nc.sync.dma_start(w1_sb[:], w1_t[:])
            wmv_sb = cpool.tile([hid_dim, lat2], bf16)
            nc.sync.dma_start(wmv_sb[:], wmv_t[:])
            dinv_sb = cpool.tile([128, tiles_pc], f32)
            nc.sync.dma_start(dinv_sb[:], dinv_t[:])
            idx_sb = cpool.tile([128, p.n_slots // 16], mybir.dt.int16)
            nc.sync.dma_start(idx_sb[:], idx_t[:])
            noise_sb = cpool.tile([128, tiles_pc * lat_dim], f32)
            nc.sync.dma_start(noise_sb[:], noise_t[:])
            if has_b1:
                b1b_sb = cpool.tile([128, hid_dim], f32)
                nc.sync.dma_start(b1b_sb[:], b1b_t[:])
            if has_bmv:
                bmvb_sb = cpool.tile([128, lat2], f32)
                nc.sync.dma_start(bmvb_sb[:], bmvb_t[:])

            xw_own_sb = cpool.tile([128, tiles_pc * hid_dim], bf16)
            hw2_own_sb = cpool.tile([128, tiles_pc * lat2], bf16)
            out_sb = cpool.tile([128, tiles_pc * 3 * lat_dim], f32)

            if no_collective:
                collective_mode = "none"
            addr = ("Shared" if collective_mode.endswith("shared") else "Local")
            chunked = collective_mode.startswith("chunked")
            xw_shard = dram.tile([dpc, ROW], bf16)
            hw2_shard = dram.tile([dpc, ROW], bf16)
            xw_full = nc.dram_tensor("xw_full", [p.npad, ROW], bf16,
                                     kind="Internal", addr_space=addr)
            hw2_full = nc.dram_tensor("hw2_full", [p.npad, ROW], bf16,
                                      kind="Internal", addr_space=addr)

            def emit_allgather(shard, full, q):
                if collective_mode == "none":
                    return
                if not chunked:
                    if q != p.n_chunks - 1:
                        return
                    ins, outs = [shard[:]], [full[:]]
                else:
                    ins = [shard[q * qsize:(q + 1) * qsize, :]]
                    outs = [full[q * p.chunk:(q + 1) * p.chunk, :]]
                nc.gpsimd.collective_compute(
                    "AllGather",
                    mybir.AluOpType.bypass,
                    replica_groups=[list(range(p.cores))],
                    ins=ins,
                    outs=outs,
                )

            def store_supertile(shard, own_sb, s, width):
                """DMA supertile s's rows of own_sb into 256B-strided shard."""
                t0 = s * st_tiles
                nt = min(st_tiles, tiles_pc - t0)
                src = own_sb[:, t0 * width:(t0 + nt) * width]
                src3 = src.rearrange("p (t e) -> p t e", t=nt)
                dst = shard[t0 * TILE_D:(t0 + nt) * TILE_D, :width]
                dst3 = dst.rearrange("(t p) e -> p t e", t=nt)
                nc.sync.dma_start(dst3, src3)

            # ---------------- phase A: xw = (dinv*x) @ W1 ----------------
            for s in range(n_st):
                t0 = s * st_tiles
                nt = min(st_tiles, tiles_pc - t0)
                xT_sl = ep.tile([128, st_tiles * TILE_D], bf16, tag="xT")
                nc.sync.dma_start(xT_sl[:, :nt * TILE_D],
                                  xT_t[:, t0 * TILE_D:(t0 + nt) * TILE_D])
                for tl in range(nt):
                    t = t0 + tl
                    xw_ps = pse.tile([128, hid_dim], f32, tag="eo")
                    nc.tensor.matmul(
                        xw_ps[:], xT_sl[:, tl * TILE_D:(tl + 1) * TILE_D],
                        w1_sb[:], start=True, stop=True)
                    nc.scalar.copy(xw_own_sb[:, t * hid_dim:(t + 1) * hid_dim],
                                   xw_ps[:])
                store_supertile(xw_shard, xw_own_sb, s, hid_dim)
                for q in range(p.n_chunks):
                    if fin_st[q] == s:
                        emit_allgather(xw_shard, xw_full, q)

            # ---------------- aggregation pass scaffold ----------------
            def agg_pass(src_full, epilogue, width):
                for s in range(n_st):
                    t0 = s * st_tiles
                    nt = min(st_tiles, tiles_pc - t0)
                    agg_ps = ps.tile([128, nt * BANK_F32], f32, tag="agg")
                    for seg in segs_by_st[s]:
                        cc = seg["chunk"]
                        ng_seg = seg["g_hi"] - seg["g_lo"]
                        if ng_seg == 0:
                            continue
                        s_sb = sb.tile([128, ng_seg * 32], bf16, tag="sbf")
                        nc.sync.dma_start(
                            s_sb[:],
                            sbf_t[:, seg["g_lo"] * 32:seg["g_hi"] * 32])
                        for ga in range(seg["g_lo"], seg["g_hi"], GMAX):
                            gb = min(ga + GMAX, seg["g_hi"])
                            ng = gb - ga
                            nsl = ng * K
                            sa = seg["s_lo"] + (ga - seg["g_lo"]) * K
                            msgs = sb.tile([128, GMAX * ROW], bf16, tag="msgs")
                            msgs3 = msgs[:, :ng * ROW].rearrange(
                                "p (g e) -> p g e", g=ng)
                            r0 = cc * p.chunk
                            nc.gpsimd.dma_gather(
                                msgs3, src_full[r0:r0 + p.chunk, :],
                                idx_sb[:, sa // 16:(sa + nsl) // 16],
                                nsl, nsl, ROW)
                            for g in range(ga, gb):
                                gq = g - ga
                                tl = int(p.group_tile[g]) - t0
                                ww = int(p.group_win[g])
                                nc.tensor.matmul(
                                    agg_ps[32 * ww:32 * (ww + 1),
                                           tl * BANK_F32:tl * BANK_F32 + width],
                                    s_sb[:, (g - seg["g_lo"]) * 32:
                                         (g - seg["g_lo"] + 1) * 32],
                                    msgs3[:, gq, :width],
                                    start=bool(p.group_first[g]),
                                    stop=bool(p.group_last[g]),
                                    tile_position=(0, 32 * ww),
                                    skip_group_check=True,
                                )
                    for tl in range(nt):
                        epilogue(t0 + tl,
                                 agg_ps[:, tl * BANK_F32:tl * BANK_F32 + width])

            # ---------------- pass 1: conv1 -> hw2 ----------------
            def epi1(t, agg_slice):
                dv = dinv_sb[:, t:t + 1]
                hcols = slice(t * hid_dim, (t + 1) * hid_dim)
                pre = ep.tile([128, hid_dim], f32, tag="e1pre")
                nc.vector.tensor_tensor(pre[:], agg_slice,
                                        xw_own_sb[:, hcols],
                                        mybir.AluOpType.add)
                hs_sb = ep.tile([128, hid_dim], f32, tag="e1hs")
                if has_b1:
                    tmp = ep.tile([128, hid_dim], f32, tag="e1tmp")
                    nc.vector.tensor_scalar(tmp[:], pre[:], dv, None,
                                            mybir.AluOpType.mult)
                    nc.vector.tensor_tensor(tmp[:], tmp[:], b1b_sb[:],
                                            mybir.AluOpType.add)
                    nc.scalar.activation(hs_sb[:], tmp[:],
                                         mybir.ActivationFunctionType.Tanh)
                else:
                    nc.scalar.activation(hs_sb[:], pre[:],
                                         mybir.ActivationFunctionType.Tanh,
                                         scale=dv)
                nc.vector.tensor_scalar(hs_sb[:], hs_sb[:], dv, None,
                                        mybir.AluOpType.mult)
                hsT_ps = pse.tile([128, 128], f32, tag="eT")
                nc.tensor.transpose(hsT_ps[:hid_dim, :], hs_sb[:], ident[:])
                hsT_sb = ep.tile([hid_dim, 128], bf16, tag="e1Ts")
                nc.scalar.copy(hsT_sb[:], hsT_ps[:hid_dim, :])
                hw2_ps = pse.tile([128, lat2], f32, tag="eo")
                nc.tensor.matmul(hw2_ps[:], hsT_sb[:], wmv_sb[:],
                                 start=True, stop=True)
                nc.scalar.copy(hw2_own_sb[:, t * lat2:(t + 1) * lat2],
                               hw2_ps[:])

            agg_pass(xw_full, epi1, hid_dim)
            # flush hw2 shard + chunked AllGathers as supertiles complete
            # (emitted after the pass; deps keyed on hw2_own_sb regions)
            for s in range(n_st):
                store_supertile(hw2_shard, hw2_own_sb, s, lat2)
                for q in range(p.n_chunks):
                    if fin_st[q] == s:
                        emit_allgather(hw2_shard, hw2_full, q)

            # ---------------- pass 2: conv2/3 -> mean/logvar/z ----------------
            def epi2(t, agg_slice):
                dv = dinv_sb[:, t:t + 1]
                ob = t * 3 * lat_dim
                mlv = ep.tile([128, lat2], f32, tag="e2mlv")
                nc.vector.tensor_tensor(mlv[:], agg_slice,
                                        hw2_own_sb[:, t * lat2:(t + 1) * lat2],
                                        mybir.AluOpType.add)
                nc.vector.tensor_scalar(out_sb[:, ob:ob + lat2], mlv[:], dv,
                                        None, mybir.AluOpType.mult)
                if has_bmv:
                    nc.vector.tensor_tensor(out_sb[:, ob:ob + lat2],
                                            out_sb[:, ob:ob + lat2],
                                            bmvb_sb[:], mybir.AluOpType.add)
                ev = ep.tile([128, lat_dim], f32, tag="e2ev")
                nc.scalar.activation(ev[:], out_sb[:, ob + lat_dim:ob + lat2],
                                     mybir.ActivationFunctionType.Exp,
                                     scale=0.5)
                zcols = slice(ob + lat2, ob + 3 * lat_dim)
                nc.vector.tensor_tensor(
                    out_sb[:, zcols], ev[:],
                    noise_sb[:, t * lat_dim:(t + 1) * lat_dim],
                    mybir.AluOpType.mult)
                nc.vector.tensor_tensor(out_sb[:, zcols], out_sb[:, zcols],
                                        out_sb[:, ob:ob + lat_dim],
                                        mybir.AluOpType.add)

            agg_pass(hw2_full, epi2, lat2)
            nc.sync.dma_start(out_t[:], out_sb[:])

    nc.compile()
    return nc


# ============================================================ host driver
def make_inputs(p, x, W1, b1, Wm, bm, Wv, bv, noise, in_dim, hid_dim, lat_dim):
    import ml_dtypes
    bf16 = ml_dtypes.bfloat16

    n = x.shape[0]
    xs = np.zeros((p.npad, in_dim), np.float32)
    xs[:n] = x * p.dinv[:n, None]
    noise_pad = np.zeros((p.npad, lat_dim), np.float32)
    noise_pad[:n] = noise
    wmv = np.concatenate([Wm, Wv], axis=1).astype(bf16)
    w1b = np.asarray(W1, bf16)
    b1b = np.tile(np.asarray(b1, np.float32)[None, :], (128, 1))
    bmvb = np.tile(np.concatenate([bm, bv]).astype(np.float32)[None, :],
                   (128, 1))
    lat2 = 2 * lat_dim

    in_maps = []
    for cc in range(p.cores):
        rows = slice(cc * p.dpc, (cc + 1) * p.dpc)
        dv = p.dinv[rows]
        noise_r = noise_pad[rows].reshape(p.tiles_pc, TILE_D, lat_dim)
        noise_r = noise_r.transpose(1, 0, 2).reshape(128, -1)
        m = {
            "xT": np.ascontiguousarray(xs[rows].T).astype(bf16),
            "w1": w1b,
            "wmv": wmv,
            "idx": p.idx_wrapped[cc],
            "sbf": np.ascontiguousarray(p.s8[cc]).astype(bf16),
            "dinv": np.ascontiguousarray(
                dv.reshape(p.tiles_pc, TILE_D).T).astype(np.float32),
            "noise": np.ascontiguousarray(noise_r).astype(np.float32),
        }
        if np.any(b1b != 0):
            m["b1b"] = b1b
        if np.any(bmvb != 0):
            m["bmvb"] = bmvb
        in_maps.append(m)
    return in_maps


def unpack_outputs(p, results, n, lat_dim):
    zs, means, logvars = [], [], []
    for cc in range(p.cores):
        o = results[cc]["out"].reshape(128, p.tiles_pc, 3 * lat_dim)
        o = o.transpose(1, 0, 2).reshape(p.dpc, 3 * lat_dim)
        means.append(o[:, :lat_dim])
        logvars.append(o[:, lat_dim:2 * lat_dim])
        zs.append(o[:, 2 * lat_dim:])
    z = np.concatenate(zs, axis=0)[:n]
    mean = np.concatenate(means, axis=0)[:n]
    logvar = np.concatenate(logvars, axis=0)[:n]
    return z, mean, logvar


def prepare(x, edge_index, W1, b1, Wm, bm, Wv, bv, noise,
            cores=N_CORES, st_tiles=ST_TILES,
            collective_mode="chunked_shared"):
    n, in_dim = x.shape
    hid_dim = W1.shape[1]
    lat_dim = Wm.shape[1]
    p = preprocess(np.asarray(edge_index, np.int64), n, cores,
                   st_tiles=st_tiles)
    has_b1 = bool(np.any(np.asarray(b1) != 0))
    has_bmv = bool(np.any(np.asarray(bm) != 0) or np.any(np.asarray(bv) != 0))
    nc = build_program(p, in_dim, hid_dim, lat_dim, has_b1, has_bmv,
                       collective_mode=collective_mode)
    in_maps = make_inputs(p, np.asarray(x, np.float32), W1, b1, Wm, bm, Wv, bv,
                          np.asarray(noise, np.float32),
                          in_dim, hid_dim, lat_dim)
    return nc, in_maps, p


def run(x, edge_index, W1, b1, Wm, bm, Wv, bv, noise,
        cores=N_CORES, st_tiles=ST_TILES, trace=False):
    from concourse.bass_utils import run_bass_kernel_spmd

    n = x.shape[0]
    lat_dim = Wm.shape[1]
    nc, in_maps, p = prepare(x, edge_index, W1, b1, Wm, bm, Wv, bv, noise,
                             cores=cores, st_tiles=st_tiles)
    res = run_bass_kernel_spmd(nc, in_maps, core_ids=list(range(cores)),
                               trace=trace)
    z, mean, logvar = unpack_outputs(p, res.results, n, lat_dim)
    return (z, mean, logvar), res


def kernel(x, edge_index, W1, b1, Wm, bm, Wv, bv, noise):
    (z, mean, logvar), _ = run(np.asarray(x), np.asarray(edge_index),
                               np.asarray(W1), np.asarray(b1),
                               np.asarray(Wm), np.asarray(bm),
                               np.asarray(Wv), np.asarray(bv),
                               np.asarray(noise))
    return (z, mean, logvar)


# revision 26
# speedup vs baseline: 1.0749x; 1.0749x over previous
"""Trainium2 Bass kernel for the PRADA GCN encoder (3x GCNConv message passing).

Math (matching the jax reference):
    src/dst = edges + self loops;  deg = indegree(dst);  dinv = rsqrt(deg)
    conv(x, W, b) = dinv_d * ((sum_{e: dst=d} dinv_src * x_src) @ W) + b
    h      = tanh(conv(x, W1, b1))
    mean   = conv(h, Wm, bm);  logvar = conv(h, Wv, bv)
    z      = noise * exp(0.5*logvar) + mean

Strategy (8 NeuronCores, single SPMD NEFF), v2 "project-first":
  - Both convs are computed as  aggregate(project(x)):  the projected node
    features (xw = (dinv*x)@W1 [64],  hw2 = hs@[Wm|Wv] [64]) are what gets
    gathered per edge, as bf16 rows padded to 256B (the 128B pad is never
    read by the aggregation matmuls).
  - Destination nodes sharded contiguously across cores (12544 rows/core).
    Edges grouped host-side into K=128-slot groups; each group targets one
    32-destination PSUM window of one 128-dst tile via a one-hot bf16
    S [128, 32] stationary matmul accumulating in PSUM.
  - Gathers use gpsimd.dma_gather (int16 indices) over the 4 quarters of a
    permuted global row space; each quarter is produced by its own chunked
    AllGather (Shared output) so collectives overlap the producing pass.
  - Uniform group skeleton across cores (max over the 8 cores) so one NEFF
    serves all cores; per-core variation lives in idx / S input data.
  - Epilogues keep everything SBUF-resident: noise preloaded once, outputs
    accumulated in SBUF and stored once, per-supertile DRAM stores for the
    collective inputs.
"""

import numpy as np

# ----------------------------------------------------------------- constants
TILE_D = 128     # destinations per tile (PSUM partition dim)
WIN = 32         # destinations per PSUM window (col-group granularity)
NWIN = TILE_D // WIN
K = 128          # slots per group (matmul contraction dim)

# problem config (graded problem; kernel.py must be self-contained)
N_NODES = 100000
N_EDGES = 1200000
IN_DIM, HID_DIM, LAT_DIM = 128, 64, 32
N_CORES = 8
GMAX = 8         # max groups (1024 idxs) per dma_gather call
DMA_SCRATCH = 2**14  # SWDGE ring: 1024 descs (16B each, per partition)
N_SWDGE_QUEUES = 4   # spread gathers over SWDGE queues (desc-rate bound)
BANK_F32 = 512   # fp32 elements per 2KB PSUM bank; matmul start=True zeroes
                 # the whole bank (ZERO_REGION) => one agg tile per bank
ST_TILES = 6     # tiles per supertile (each agg tile owns a PSUM bank)
N_CHUNKS = 4     # src-space quarters (int16 gather index limit)
ROW = 128        # padded feature row: 128 bf16 = 256B (dma_gather minimum)


def _ceil_div(a, b):
    return -(-a // b)


# ============================================================ preprocessing
class Prep:
    pass


def preprocess(edge_index, n_nodes, cores, st_tiles=ST_TILES):
    """Build the uniform group skeleton + per-core idx/S tensors."""
    p = Prep()
    src = edge_index[0].astype(np.int64)
    dst = edge_index[1].astype(np.int64)

    npad = _ceil_div(n_nodes, cores * TILE_D) * cores * TILE_D
    dpc = npad // cores
    tiles_pc = dpc // TILE_D
    n_chunks = N_CHUNKS
    qsize = dpc // n_chunks          # rows per (core, quarter)
    assert dpc % n_chunks == 0
    chunk = qsize * cores            # rows per permuted global quarter
    assert chunk <= 32767
    n_st = _ceil_div(tiles_pc, st_tiles)

    # degree includes the self loop (+1); pad nodes get deg 1
    deg = np.bincount(dst, minlength=npad).astype(np.float32)
    deg[:n_nodes] += 1.0
    deg[n_nodes:] = 1.0
    dinv = (1.0 / np.sqrt(deg)).astype(np.float32)

    core_id = dst // dpc
    dloc = dst % dpc
    t = dloc // TILE_D
    w = (dloc % TILE_D) // WIN
    wcol = dloc % WIN
    # permuted src space: row r = dpc*k + m, m = 3136*q + j
    #   -> quarter q, local idx 3136*k + j  (quarter q = AllGather q's output)
    sk = src // dpc
    sm = src % dpc
    c = sm // qsize
    srcl = (qsize * sk + (sm % qsize)).astype(np.int16)

    # counts per (core, t, w, c) -> uniform skeleton G = ceil(max_cores/K)
    key = ((core_id * tiles_pc + t) * NWIN + w) * n_chunks + c
    counts = np.bincount(key, minlength=cores * tiles_pc * NWIN * n_chunks)
    counts = counts.reshape(cores, tiles_pc, NWIN, n_chunks)
    G = _ceil_div(counts.max(axis=0), K).astype(np.int64)  # [tiles_pc, NWIN, n_chunks]
    # every (t, w) needs >=1 group so its PSUM window gets written (start=True)
    empty_tw = G.sum(axis=2) == 0
    G[:, :, 0][empty_tw] = 1

    # enumerate groups in emission order: st -> chunk -> tile -> window -> g
    slot_off = np.zeros((tiles_pc, NWIN, n_chunks), np.int64)
    group_tile = []
    group_win = []
    segments = []  # per (st, c): dict
    gidx = 0
    soff = 0
    for s in range(n_st):
        ts = range(s * st_tiles, min((s + 1) * st_tiles, tiles_pc))
        for cc in range(n_chunks):
            g_lo, s_lo = gidx, soff
            for tt in ts:
                for ww in range(NWIN):
                    gg = int(G[tt, ww, cc])
                    slot_off[tt, ww, cc] = soff
                    group_tile.extend([tt] * gg)
                    group_win.extend([ww] * gg)
                    gidx += gg
                    soff += gg * K
            segments.append(dict(st=s, chunk=cc, g_lo=g_lo, g_hi=gidx,
                                 s_lo=s_lo, s_hi=soff))
    n_groups, n_slots = gidx, soff
    group_tile = np.asarray(group_tile, np.int64)
    group_win = np.asarray(group_win, np.int64)

    # first/last group flags per (t, w) in emission order
    first = np.zeros(n_groups, bool)
    last = np.zeros(n_groups, bool)
    seen = {}
    for g in range(n_groups):
        kk = (int(group_tile[g]), int(group_win[g]))
        if kk not in seen:
            first[g] = True
        seen[kk] = g
    for kk, g in seen.items():
        last[g] = True

    # ------- per-core slot assignment (vectorized rank within (core,t,w,c))
    order = np.lexsort((c, w, t, core_id))
    key_sorted = key[order]
    starts = np.r_[0, np.flatnonzero(np.diff(key_sorted)) + 1]
    group_start_of = np.zeros(len(key_sorted), np.int64)
    group_start_of[starts] = starts
    np.maximum.accumulate(group_start_of, out=group_start_of)
    rank = np.arange(len(key_sorted)) - group_start_of

    slot = slot_off[t[order], w[order], c[order]] + rank
    corev = core_id[order]

    idx16 = np.zeros((cores, n_slots), np.int16)
    idx16[corev, slot] = srcl[order]
    s8 = np.zeros((cores, 128, n_groups * 32), np.int8)
    s8[corev, slot % K, (slot // K) * 32 + wcol[order]] = 1

    # wrapped gather-index layout: idx i -> [i%16, i//16], replicated x8 rows
    wrapped = idx16.reshape(cores, -1, 16).transpose(0, 2, 1)  # [cores,16,S/16]
    wrapped = np.tile(wrapped, (1, 8, 1)).copy()               # [cores,128,S/16]

    p.npad, p.dpc, p.tiles_pc, p.n_st = npad, dpc, tiles_pc, n_st
    p.n_chunks, p.chunk, p.qsize, p.st_tiles = n_chunks, chunk, qsize, st_tiles
    p.n_groups, p.n_slots = n_groups, n_slots
    p.group_tile, p.group_win = group_tile, group_win
    p.group_first, p.group_last = first, last
    p.segments = segments
    p.dinv = dinv
    p.idx_wrapped = wrapped
    p.s8 = s8
    p.cores = cores
    return p


# ============================================================ bass program
def build_program(p, in_dim, hid_dim, lat_dim, has_b1, has_bmv,
                  no_collective=False, collective_mode="chunked_shared"):
    import concourse.bacc as bacc
    import concourse.mybir as mybir
    import concourse.tile as tile
    from concourse.library_config import mlp
    from concourse.masks import make_identity

    f32 = mybir.dt.float32
    bf16 = mybir.dt.bfloat16
    nc = bacc.Bacc("TRN2", target_bir_lowering=False, debug=False,
                   num_devices=p.cores, dynamic_dma_scratch_size=DMA_SCRATCH,
                   num_swdge_queues=N_SWDGE_QUEUES)

    lat2 = 2 * lat_dim
    n_st, st_tiles, tiles_pc = p.n_st, p.st_tiles, p.tiles_pc
    dpc, qsize = p.dpc, p.qsize

    # ---- inputs
    xT_t = nc.dram_tensor("xT", [in_dim, dpc], bf16, kind="ExternalInput")
    w1_t = nc.dram_tensor("w1", [in_dim, hid_dim], bf16, kind="ExternalInput")
    wmv_t = nc.dram_tensor("wmv", [hid_dim, lat2], bf16, kind="ExternalInput")
    idx_t = nc.dram_tensor("idx", [128, p.n_slots // 16], mybir.dt.int16,
                           kind="ExternalInput")
    sbf_t = nc.dram_tensor("sbf", [128, p.n_groups * 32], bf16,
                           kind="ExternalInput")
    dinv_t = nc.dram_tensor("dinv", [128, tiles_pc], f32, kind="ExternalInput")
    noise_t = nc.dram_tensor("noise", [128, tiles_pc * lat_dim], f32,
                             kind="ExternalInput")
    if has_b1:
        b1b_t = nc.dram_tensor("b1b", [128, hid_dim], f32, kind="ExternalInput")
    if has_bmv:
        bmvb_t = nc.dram_tensor("bmvb", [128, lat2], f32, kind="ExternalInput")

    # ---- output: per tile block [mean(32) | logvar(32) | z(32)]
    out_t = nc.dram_tensor("out", [128, tiles_pc * 3 * lat_dim], f32,
                           kind="ExternalOutput")

    segs_by_st = {}
    for seg in p.segments:
        segs_by_st.setdefault(seg["st"], []).append(seg)

    # quarter q's shard rows are complete after supertile fin_st[q]
    fin_st = [min(_ceil_div((q + 1) * qsize, st_tiles * TILE_D), n_st) - 1
              for q in range(p.n_chunks)]

    with tile.TileContext(nc) as tc:
        with (
            tc.tile_pool(name="const", bufs=1) as cpool,
            tc.tile_pool(name="sb", bufs=2) as sb,
            tc.tile_pool(name="ep", bufs=3) as ep,
            tc.tile_pool(name="ps", bufs=1, space="PSUM") as ps,
            tc.tile_pool(name="pse", bufs=1, space="PSUM") as pse,
            tc.tile_pool(name="dram", bufs=1, space="DRAM") as dram,
        ):
            nc.gpsimd.load_library(mlp)
            ident = cpool.tile([128, 128], f32)
            make_identity(nc, ident[:])
            w1_sb = cpool.tile([in_dim, hid_dim], bf16)
            nc.sync.dma_start(w1_sb[:], w1_t[:])
            wmv_sb = cpool.tile([hid_dim, lat2], bf16)
            nc.sync.dma_start(wmv_sb[:], wmv_t[:])
            dinv_sb = cpool.tile([128, tiles_pc], f32)
            nc.sync.dma_start(dinv_sb[:], dinv_t[:])
            idx_sb = cpool.tile([128, p.n_slots // 16], mybir.dt.int16)
            nc.sync.dma_start(idx_sb[:], idx_t[:])
            noise_sb = cpool.tile([128, tiles_pc * lat_dim], f32)
            nc.sync.dma_start(noise_sb[:], noise_t[:])
            if has_b1:
                b1b_sb = cpool.tile([128, hid_dim], f32)
                nc.sync.dma_start(b1b_sb[:], b1b_t[:])
            if has_bmv:
                bmvb_sb = cpool.tile([128, lat2], f32)
                nc.sync.dma_start(bmvb_sb[:], bmvb_t[:])

            xw_own_sb = cpool.tile([128, tiles_pc * hid_dim], bf16)
            hw2_own_sb = cpool.tile([128, tiles_pc * lat2], bf16)
            out_sb = cpool.tile([128, tiles_pc * 3 * lat_dim], f32)

            if no_collective:
                collective_mode = "none"
            addr = ("Shared" if collective_mode.endswith("shared") else "Local")
            chunked = collective_mode.startswith("chunked")
            xw_shard = dram.tile([dpc, ROW], bf16)
            hw2_shard = dram.tile([dpc, ROW], bf16)
            xw_full = nc.dram_tensor("xw_full", [p.npad, ROW], bf16,
                                     kind="Internal", addr_space=addr)
            hw2_full = nc.dram_tensor("hw2_full", [p.npad, ROW], bf16,
                                      kind="Internal", addr_space=addr)

            def emit_allgather(shard, full, q):
                if collective_mode == "none":
                    return
                if not chunked:
                    if q != p.n_chunks - 1:
                        return
                    ins, outs = [shard[:]], [full[:]]
                else:
                    ins = [shard[q * qsize:(q + 1) * qsize, :]]
                    outs = [full[q * p.chunk:(q + 1) * p.chunk, :]]
                nc.gpsimd.collective_compute(
                    "AllGather",
                    mybir.AluOpType.bypass,
                    replica_groups=[list(range(p.cores))],
                    ins=ins,
                    outs=outs,
                )

            def store_supertile(shard, own_sb, s, width):
                """DMA supertile s's rows of own_sb into 256B-strided shard."""
                t0 = s * st_tiles
                nt = min(st_tiles, tiles_pc - t0)
                src = own_sb[:, t0 * width:(t0 + nt) * width]
                src3 = src.rearrange("p (t e) -> p t e", t=nt)
                dst = shard[t0 * TILE_D:(t0 + nt) * TILE_D, :width]
                dst3 = dst.rearrange("(t p) e -> p t e", t=nt)
                nc.sync.dma_start(dst3, src3)

            # ---------------- phase A: xw = (dinv*x) @ W1 ----------------
            for s in range(n_st):
                t0 = s * st_tiles
                nt = min(st_tiles, tiles_pc - t0)
                xT_sl = ep.tile([128, st_tiles * TILE_D], bf16, tag="xT")
                nc.sync.dma_start(xT_sl[:, :nt * TILE_D],
                                  xT_t[:, t0 * TILE_D:(t0 + nt) * TILE_D])
                for tl in range(nt):
                    t = t0 + tl
                    xw_ps = pse.tile([128, hid_dim], f32, tag="eo")
                    nc.tensor.matmul(
                        xw_ps[:], xT_sl[:, tl * TILE_D:(tl + 1) * TILE_D],
                        w1_sb[:], start=True, stop=True)
                    nc.scalar.copy(xw_own_sb[:, t * hid_dim:(t + 1) * hid_dim],
                                   xw_ps[:])
                store_supertile(xw_shard, xw_own_sb, s, hid_dim)
                for q in range(p.n_chunks):
                    if fin_st[q] == s:
                        emit_allgather(xw_shard, xw_full, q)

            # ---------------- aggregation pass scaffold ----------------
            def agg_pass(src_full, epilogue, width):
                call_no = 0
                for s in range(n_st):
                    t0 = s * st_tiles
                    nt = min(st_tiles, tiles_pc - t0)
                    agg_ps = ps.tile([128, nt * BANK_F32], f32, tag="agg")
                    for seg in segs_by_st[s]:
                        cc = seg["chunk"]
                        ng_seg = seg["g_hi"] - seg["g_lo"]
                        if ng_seg == 0:
                            continue
                        s_sb = sb.tile([128, ng_seg * 32], bf16, tag="sbf")
                        nc.sync.dma_start(
                            s_sb[:],
                            sbf_t[:, seg["g_lo"] * 32:seg["g_hi"] * 32])
                        for ga in range(seg["g_lo"], seg["g_hi"], GMAX):
                            gb = min(ga + GMAX, seg["g_hi"])
                            ng = gb - ga
                            nsl = ng * K
                            sa = seg["s_lo"] + (ga - seg["g_lo"]) * K
                            msgs = sb.tile([128, GMAX * ROW], bf16, tag="msgs")
                            msgs3 = msgs[:, :ng * ROW].rearrange(
                                "p (g e) -> p g e", g=ng)
                            r0 = cc * p.chunk
                            nc.gpsimd.dma_gather(
                                msgs3, src_full[r0:r0 + p.chunk, :],
                                idx_sb[:, sa // 16:(sa + nsl) // 16],
                                nsl, nsl, ROW,
                                queue_num=call_no % N_SWDGE_QUEUES)
                            call_no += 1
                            for g in range(ga, gb):
                                gq = g - ga
                                tl = int(p.group_tile[g]) - t0
                                ww = int(p.group_win[g])
                                nc.tensor.matmul(
                                    agg_ps[32 * ww:32 * (ww + 1),
                                           tl * BANK_F32:tl * BANK_F32 + width],
                                    s_sb[:, (g - seg["g_lo"]) * 32:
                                         (g - seg["g_lo"] + 1) * 32],
                                    msgs3[:, gq, :width],
                                    start=bool(p.group_first[g]),
                                    stop=bool(p.group_last[g]),
                                    tile_position=(0, 32 * ww),
                                    skip_group_check=True,
                                )
                    for tl in range(nt):
                        epilogue(t0 + tl,
                                 agg_ps[:, tl * BANK_F32:tl * BANK_F32 + width])

            # ---------------- pass 1: conv1 -> hw2 ----------------
            def epi1(t, agg_slice):
                dv = dinv_sb[:, t:t + 1]
                hcols = slice(t * hid_dim, (t + 1) * hid_dim)
                pre = ep.tile([128, hid_dim], f32, tag="e1pre")
                nc.vector.tensor_tensor(pre[:], agg_slice,
                                        xw_own_sb[:, hcols],
                                        mybir.AluOpType.add)
                hs_sb = ep.tile([128, hid_dim], f32, tag="e1hs")
                if has_b1:
                    tmp = ep.tile([128, hid_dim], f32, tag="e1tmp")
                    nc.vector.tensor_scalar(tmp[:], pre[:], dv, None,
                                            mybir.AluOpType.mult)
                    nc.vector.tensor_tensor(tmp[:], tmp[:], b1b_sb[:],
                                            mybir.AluOpType.add)
                    nc.scalar.activation(hs_sb[:], tmp[:],
                                         mybir.ActivationFunctionType.Tanh)
                else:
                    nc.scalar.activation(hs_sb[:], pre[:],
                                         mybir.ActivationFunctionType.Tanh,
                                         scale=dv)
                nc.vector.tensor_scalar(hs_sb[:], hs_sb[:], dv, None,
                                        mybir.AluOpType.mult)
                hsT_ps = pse.tile([128, 128], f32, tag="eT")
                nc.tensor.transpose(hsT_ps[:hid_dim, :], hs_sb[:], ident[:])
                hsT_sb = ep.tile([hid_dim, 128], bf16, tag="e1Ts")
                nc.scalar.copy(hsT_sb[:], hsT_ps[:hid_dim, :])
                hw2_ps = pse.tile([128, lat2], f32, tag="eo")
                nc.tensor.matmul(hw2_ps[:], hsT_sb[:], wmv_sb[:],
                                 start=True, stop=True)
                nc.scalar.copy(hw2_own_sb[:, t * lat2:(t + 1) * lat2],
                               hw2_ps[:])

            agg_pass(xw_full, epi1, hid_dim)
            # flush hw2 shard + chunked AllGathers as supertiles complete
            # (emitted after the pass; deps keyed on hw2_own_sb regions)
            for s in range(n_st):
                store_supertile(hw2_shard, hw2_own_sb, s, lat2)
                for q in range(p.n_chunks):
                    if fin_st[q] == s:
                        emit_allgather(hw2_shard, hw2_full, q)

            # ---------------- pass 2: conv2/3 -> mean/logvar/z ----------------
            def epi2(t, agg_slice):
                dv = dinv_sb[:, t:t + 1]
                ob = t * 3 * lat_dim
                mlv = ep.tile([128, lat2], f32, tag="e2mlv")
                nc.vector.tensor_tensor(mlv[:], agg_slice,
                                        hw2_own_sb[:, t * lat2:(t + 1) * lat2],
                                        mybir.AluOpType.add)
                nc.vector.tensor_scalar(out_sb[:, ob:ob + lat2], mlv[:], dv,
                                        None, mybir.AluOpType.mult)
                if has_bmv:
                    nc.vector.tensor_tensor(out_sb[:, ob:ob + lat2],
                                            out_sb[:, ob:ob + lat2],
                                            bmvb_sb[:], mybir.AluOpType.add)
                ev = ep.tile([128, lat_dim], f32, tag="e2ev")
                nc.scalar.activation(ev[:], out_sb[:, ob + lat_dim:ob + lat2],
                                     mybir.ActivationFunctionType.Exp,
                                     scale=0.5)
                zcols = slice(ob + lat2, ob + 3 * lat_dim)
                nc.vector.tensor_tensor(
                    out_sb[:, zcols], ev[:],
                    noise_sb[:, t * lat_dim:(t + 1) * lat_dim],
                    mybir.AluOpType.mult)
                nc.vector.tensor_tensor(out_sb[:, zcols], out_sb[:, zcols],
                                        out_sb[:, ob:ob + lat_dim],
                                        mybir.AluOpType.add)

            agg_pass(hw2_full, epi2, lat2)
            nc.sync.dma_start(out_t[:], out_sb[:])

    nc.compile()
    return nc


# ============================================================ host driver
def make_inputs(p, x, W1, b1, Wm, bm, Wv, bv, noise, in_dim, hid_dim, lat_dim):
    import ml_dtypes
    bf16 = ml_dtypes.bfloat16

    n = x.shape[0]
    xs = np.zeros((p.npad, in_dim), np.float32)
    xs[:n] = x * p.dinv[:n, None]
    noise_pad = np.zeros((p.npad, lat_dim), np.float32)
    noise_pad[:n] = noise
    wmv = np.concatenate([Wm, Wv], axis=1).astype(bf16)
    w1b = np.asarray(W1, bf16)
    b1b = np.tile(np.asarray(b1, np.float32)[None, :], (128, 1))
    bmvb = np.tile(np.concatenate([bm, bv]).astype(np.float32)[None, :],
                   (128, 1))
    lat2 = 2 * lat_dim

    in_maps = []
    for cc in range(p.cores):
        rows = slice(cc * p.dpc, (cc + 1) * p.dpc)
        dv = p.dinv[rows]
        noise_r = noise_pad[rows].reshape(p.tiles_pc, TILE_D, lat_dim)
        noise_r = noise_r.transpose(1, 0, 2).reshape(128, -1)
        m = {
            "xT": np.ascontiguousarray(xs[rows].T).astype(bf16),
            "w1": w1b,
            "wmv": wmv,
            "idx": p.idx_wrapped[cc],
            "sbf": np.ascontiguousarray(p.s8[cc]).astype(bf16),
            "dinv": np.ascontiguousarray(
                dv.reshape(p.tiles_pc, TILE_D).T).astype(np.float32),
            "noise": np.ascontiguousarray(noise_r).astype(np.float32),
        }
        if np.any(b1b != 0):
            m["b1b"] = b1b
        if np.any(bmvb != 0):
            m["bmvb"] = bmvb
        in_maps.append(m)
    return in_maps


def unpack_outputs(p, results, n, lat_dim):
    zs, means, logvars = [], [], []
    for cc in range(p.cores):
        o = results[cc]["out"].reshape(128, p.tiles_pc, 3 * lat_dim)
        o = o.transpose(1, 0, 2).reshape(p.dpc, 3 * lat_dim)
        means.append(o[:, :lat_dim])
        logvars.append(o[:, lat_dim:2 * lat_dim])
        zs.append(o[:, 2 * lat_dim:])
    z = np.concatenate(zs, axis=0)[:n]
    mean = np.concatenate(means, axis=0)[:n]
    logvar = np.concatenate(logvars, axis=0)[:n]
    return z, mean, logvar


def prepare(x, edge_index, W1, b1, Wm, bm, Wv, bv, noise,
            cores=N_CORES, st_tiles=ST_TILES,
            collective_mode="chunked_shared"):
    n, in_dim = x.shape
    hid_dim = W1.shape[1]
    lat_dim = Wm.shape[1]
    p = preprocess(np.asarray(edge_index, np.int64), n, cores,
                   st_tiles=st_tiles)
    has_b1 = bool(np.any(np.asarray(b1) != 0))
    has_bmv = bool(np.any(np.asarray(bm) != 0) or np.any(np.asarray(bv) != 0))
    nc = build_program(p, in_dim, hid_dim, lat_dim, has_b1, has_bmv,
                       collective_mode=collective_mode)
    in_maps = make_inputs(p, np.asarray(x, np.float32), W1, b1, Wm, bm, Wv, bv,
                          np.asarray(noise, np.float32),
                          in_dim, hid_dim, lat_dim)
    return nc, in_maps, p


def run(x, edge_index, W1, b1, Wm, bm, Wv, bv, noise,
        cores=N_CORES, st_tiles=ST_TILES, trace=False):
    from concourse.bass_utils import run_bass_kernel_spmd

    n = x.shape[0]
    lat_dim = Wm.shape[1]
    nc, in_maps, p = prepare(x, edge_index, W1, b1, Wm, bm, Wv, bv, noise,
                             cores=cores, st_tiles=st_tiles)
    res = run_bass_kernel_spmd(nc, in_maps, core_ids=list(range(cores)),
                               trace=trace)
    z, mean, logvar = unpack_outputs(p, res.results, n, lat_dim)
    return (z, mean, logvar), res


def kernel(x, edge_index, W1, b1, Wm, bm, Wv, bv, noise):
    (z, mean, logvar), _ = run(np.asarray(x), np.asarray(edge_index),
                               np.asarray(W1), np.asarray(b1),
                               np.asarray(Wm), np.asarray(bm),
                               np.asarray(Wv), np.asarray(bv),
                               np.asarray(noise))
    return (z, mean, logvar)
